# revision 48
# baseline (speedup 1.0000x reference)
"""GAT (3-layer, 8-head) forward on 8 Trainium2 NeuronCores.

Strategy (graph/data parallel, per sharding hint):
  - Nodes are sharded across 8 cores (2500 real -> 2560 padded rows each).
    Within a core, nodes are PERMUTED across the 20 blocks of 128 so that the
    random-edge in-degree per block is balanced (minimizes CH, the max
    128-edge chunk count per block).
  - Everything hot is bf16: GEMM operands, the allgathered feature table,
    masks, pooling.  PSUM accumulation stays fp32.
  - Per layer: h = x @ W per node block (feature-major GEMM, bf16) plus
    attention logits via an embedded (D,16) matrix WA computed on-device.
    [h | e_src] (bf16, 520 cols) is written to DRAM and AllGathered;
    e_dst (8 cols) goes to a small local DRAM table.
  - Edge phase per dst block: ONE batched indirect gather of all CH*128
    source rows from the allgathered table + one batched gather of e_dst
    rows by dst-local index.  e = leaky_relu(e_src + e_dst), ee = exp(e)
    computed block-batched.  Per chunk a 0/1 mask (generated on-chip with
    iota + is_equal from the dst-local ids) scatter-adds ee*h and ee into
    PSUM via TensorE.  Self-loop contributions are added analytically in
    the epilogue (no gather / matmul needed for them).
  - Softmax normalization after aggregation (exact up to fp reassociation;
    |e| is O(1) so max-subtraction is unnecessary).  LayerNorm uses
    bn_stats/bn_aggr on VectorE and rstd = exp(-0.5*ln(var+eps)) on ScalarE
    so the ACT LUT never leaves the exp/ln table set.
  - Mean-pool by graph via 0/1 pool-mask matmuls + AllReduce, then the FC
    head (replicated) on every core.
"""

import os
import sys

sys.path.insert(0, "/opt/trn_rl_repo")

import numpy as np
import ml_dtypes

import concourse.bass as bass
import concourse.mybir as mybir
import concourse.tile as tile
from concourse import bacc
from concourse.bass_utils import run_bass_kernel_spmd
from concourse.masks import make_identity

F32 = mybir.dt.float32
BF16 = mybir.dt.bfloat16
I32 = mybir.dt.int32
ALU = mybir.AluOpType
ACT = mybir.ActivationFunctionType
BF = ml_dtypes.bfloat16

P = 128

# Results of the last run (for test harnesses).
LAST_RESULTS = None


def _full_cfg():
    return dict(
        n_cores=8,
        N=20000,
        D=512,
        H=8,
        G=128,
        OUT=128,
        NEG=0.2,
        EPS=1e-5,
        L=3,
    )


# --------------------------------------------------------------------------
# Host-side preprocessing: index manipulation / relayout (float work limited
# to dtype casts and zero padding).
# --------------------------------------------------------------------------


def _prep(inputs, cfg):
    nc_ = cfg["n_cores"]
    N, D, H, G, OUT, L = cfg["N"], cfg["D"], cfg["H"], cfg["G"], cfg["OUT"], cfg["L"]
    C = D // H
    H2 = 2 * H

    x = np.asarray(inputs["x"], np.float32)
    ei = np.asarray(inputs["edge_index"])
    batch = np.asarray(inputs["batch"]).astype(np.int64)

    SHR = (N + nc_ - 1) // nc_  # real nodes per core
    SH = ((SHR + P - 1) // P) * P  # padded nodes per core
    NB = SH // P
    KD = D // P

    src = ei[0].astype(np.int64)
    dst = ei[1].astype(np.int64)
    scr = src // SHR  # source core
    dcr = dst // SHR  # dest core

    # --- per-core balanced node->slot permutation (by random-edge in-degree)
    perm_slot = np.zeros((nc_, SHR), np.int64)
    blk_cnt_all = np.zeros((nc_, NB), np.int64)
    for c in range(nc_):
        deg = np.bincount(dst[dcr == c] - c * SHR, minlength=SHR)
        order = np.argsort(-deg, kind="stable")
        totals = np.zeros(NB, np.int64)
        counts = np.zeros(NB, np.int64)
        slots = np.empty(SHR, np.int64)
        for n in order:
            open_b = np.nonzero(counts < P)[0]
            b = open_b[np.argmin(totals[open_b])]
            slots[n] = b * P + counts[b]
            counts[b] += 1
            totals[b] += deg[n]
        perm_slot[c] = slots
        blk_cnt_all[c] = totals

    CH = max(1, int((blk_cnt_all.max() + P - 1) // P))

    # --- edge tables, grouped by (dst core, dst block)
    spid = scr * SH + perm_slot[scr, src - scr * SHR]  # row in ag_out
    dslot = perm_slot[dcr, dst - dcr * SHR]  # local row in dst core

    gidx = np.zeros((nc_, P, NB, CH), np.int32)
    gidx16 = np.zeros((nc_, 16, NB, CH * P // 16), np.int16)
    dlix16 = np.zeros((nc_, 16, NB, CH * P // 16), np.int16)
    dlmod = np.full((nc_, P, NB, CH), -1.0, np.float32)
    for c in range(nc_):
        sel = np.nonzero(dcr == c)[0]
        ds = dslot[sel]
        b = ds // P
        order = np.argsort(b, kind="stable")
        sel, ds, b = sel[order], ds[order], b[order]
        # j = position within the block's edge list
        j = np.arange(len(sel)) - np.searchsorted(b, b)
        ch = j // P
        jj = j % P
        gidx[c, jj, b, ch] = spid[sel]
        dlmod[c, jj, b, ch] = (ds % P).astype(np.float32)
        # dma_gather int16 index layout: flat i = ch*128 + jj lives at
        # [i % 16, i // 16]; replicated over the 8 Q7 cores (128 partitions)
        i = ch * P + jj
        gidx16[c, i % 16, b, i // 16] = spid[sel].astype(np.int16)
        dlix16[c, i % 16, b, i // 16] = (c * SH + ds).astype(np.int16)
    gidx16 = np.tile(gidx16, (1, 8, 1, 1))  # [nc, 128, NB, CH*8]
    dlix16 = np.tile(dlix16, (1, 8, 1, 1))

    # --- x shards, feature-major (KD, 128, SH), bf16
    xT = np.zeros((nc_, KD, P, SH), BF)
    for c in range(nc_):
        xp = np.zeros((SH, D), np.float32)
        rows = x[c * SHR : min((c + 1) * SHR, N)]
        xp[perm_slot[c, : len(rows)]] = rows
        xT[c] = xp.T.reshape(KD, P, SH).astype(BF)

    # --- weights (bf16)
    # Feature permutation: hidden feature (h, c) is stored at column c*H + h
    # ("c-major") so per-head broadcasts hit the packed inner dim on DVE.
    # cperm[new] = old ; applied to W cols, W/A rows of the next layer,
    # b/g/be, and fc_W rows.  LayerNorm / pooling are permutation-invariant.
    cperm = (np.arange(D).reshape(H, C).T.reshape(-1)).astype(np.int64)

    W_all = np.zeros((L, KD, P, D), BF)
    WT_all = np.zeros((L, KD, P, D), BF)
    A_all = np.zeros((L, KD, P, H2), BF)
    b_l, g_l, be_l = [], [], []
    for l in range(L):
        W = np.asarray(inputs[f"W{l}"], np.float32)
        W = W[:, cperm]  # output features c-major
        if l > 0:
            W = W[cperm, :]  # input features were c-major from prev layer
        W_all[l] = W.reshape(KD, P, D).astype(BF)
        WT_all[l] = np.ascontiguousarray(W.T).reshape(KD, P, D).astype(BF)
        A = np.zeros((D, H2), np.float32)
        a_s = np.asarray(inputs[f"as{l}"], np.float32)
        a_d = np.asarray(inputs[f"ad{l}"], np.float32)
        for h in range(H):
            A[h * C : (h + 1) * C, h] = a_s[h]
            A[h * C : (h + 1) * C, H + h] = a_d[h]
        A = A[cperm, :]  # h-space rows c-major
        A_all[l] = A.reshape(KD, P, H2).astype(BF)
        b_l.append(np.asarray(inputs[f"b{l}"], np.float32)[cperm])
        g_l.append(np.asarray(inputs[f"g{l}"], np.float32)[cperm])
        be_l.append(np.asarray(inputs[f"be{l}"], np.float32)[cperm])

    skip_b = all(not b.any() for b in b_l)
    skip_g = all((g == 1.0).all() for g in g_l)
    skip_be = all(not be.any() for be in be_l)

    fc_W = (
        np.asarray(inputs["fc_W"], np.float32)[cperm, :]
        .reshape(KD, P, OUT)
        .astype(BF)
    )
    fc_b = np.asarray(inputs["fc_b"], np.float32)
    skip_fcb = not fc_b.any()

    # --- pool masks (0/1 membership, bf16), permutation-aware
    poolmask = np.zeros((nc_, NB, P, G), BF)
    for c in range(nc_):
        lo = c * SHR
        hi = min((c + 1) * SHR, N)
        sl = perm_slot[c, : hi - lo]
        poolmask[c, sl // P, sl % P, batch[lo:hi]] = 1.0

    meta = dict(
        SH=SH, NB=NB, KD=KD, CH=CH, ROW=D + H,
        skip_b=skip_b, skip_g=skip_g, skip_be=skip_be, skip_fcb=skip_fcb,
    )

    in_maps = []
    for c in range(nc_):
        m = dict(
            xT=xT[c],
            W_all=W_all,
            WT_all=WT_all,
            A_all=A_all,
            fc_W=fc_W,
            gidx16=gidx16[c],
            dlix16=dlix16[c],
            dlmod=dlmod[c],
            poolmask=poolmask[c],
        )
        if not skip_b:
            m["b_rep"] = np.broadcast_to(
                np.stack(b_l)[:, None, :], (L, P, D)
            ).copy()
        if not skip_g:
            m["g_rep"] = np.broadcast_to(
                np.stack(g_l)[:, None, :], (L, P, D)
            ).copy()
        if not skip_be:
            m["be_rep"] = np.broadcast_to(
                np.stack(be_l)[:, None, :], (L, P, D)
            ).copy()
        if not skip_fcb:
            m["fcb_rep"] = np.broadcast_to(fc_b[None, :], (P, OUT)).copy()
        in_maps.append(m)
    return in_maps, meta


# --------------------------------------------------------------------------
# Device program
# --------------------------------------------------------------------------


def build(tc, cfg, meta, I, out_ap):
    nc = tc.nc
    nc_cores = cfg["n_cores"]
    D, H, G, OUT, L = cfg["D"], cfg["H"], cfg["G"], cfg["OUT"], cfg["L"]
    NEG, EPS = cfg["NEG"], cfg["EPS"]
    C = D // H
    SH, NB, KD, CH, ROW = meta["SH"], meta["NB"], meta["KD"], meta["CH"], meta["ROW"]
    H2 = 2 * H

    rg = [list(range(nc_cores))]
    shared = "Shared" if nc_cores > 4 else "Local"
    # dma_gather needs the row stride in multiples of 256B -> 640 bf16 elems
    # row layout: [h 0:512 | e_src 512:520 | e_dst 520:528 | pad]
    RP = 640

    from contextlib import ExitStack

    ctx = ExitStack()
    res = ctx.enter_context(tc.tile_pool(name="res", bufs=1))
    dram = ctx.enter_context(tc.tile_pool(name="dram", bufs=1, space="DRAM"))
    psum = ctx.enter_context(tc.tile_pool(name="psum", bufs=1, space="PSUM"))
    sb = ctx.enter_context(tc.tile_pool(name="sb", bufs=1))

    # ---------------- resident tiles
    xT_sb = [res.tile([P, SH], BF16, name=f"xT{k}") for k in range(KD)]
    hn_sb = [res.tile([P, D], BF16, name=f"hn{b}") for b in range(NB)]
    xn_sb = [res.tile([P, D], BF16, name=f"xn{b}") for b in range(NB)]
    ybf_sb = xn_sb  # pre-norm y shares storage; norm is applied in place
    henm_sb = [res.tile([P, H2], BF16, name=f"henm{b}") for b in range(NB)]
    mv_all = res.tile([P, NB, 2], F32, name="mv_all")
    rstd_all = res.tile([P, NB], F32, name="rstd_all")
    gidx16_sb = res.tile([P, NB, CH * P // 16], mybir.dt.int16, name="gidx16")
    dlix16_sb = res.tile([P, NB, CH * P // 16], mybir.dt.int16, name="dlix16")
    dlmod_sb = res.tile([P, NB, CH], F32, name="dlmod")
    W_sb = [res.tile([P, D], BF16, name=f"W{k}") for k in range(KD)]
    WT_sb = [res.tile([P, D], BF16, name=f"WT{k}") for k in range(KD)]
    A_sb = [res.tile([P, H2], BF16, name=f"A{k}") for k in range(KD)]
    wa_sb = [res.tile([P, H2], BF16, name=f"wa{k}") for k in range(KD)]
    waT_sb = res.tile([H2, D], BF16, name="waT")
    id128 = res.tile([P, P], BF16, name="id128")
    idh2 = res.tile([H2, H2], BF16, name="idh2")
    make_identity(nc, id128[:])
    make_identity(nc, idh2[:])
    iota_sb = res.tile([P, P], F32, name="iota_sb")
    nc.gpsimd.iota(
        iota_sb[:], pattern=[[1, P]], base=0, channel_multiplier=0,
        allow_small_or_imprecise_dtypes=True,
    )
    eps_sb = res.tile([P, 1], F32, name="eps_sb")
    nc.vector.memset(eps_sb[:], float(EPS))

    # ---------------- masks are layer-invariant: generate once, keep resident
    mk_all = res.tile([P, NB, CH, P], BF16, name="mk_all")
    mkT_all = res.tile([P, NB, CH, P], BF16, name="mkT_all")

    b_rep = g_rep = be_rep = None
    if not meta["skip_b"]:
        b_rep = res.tile([P, D], F32, name="b_rep")
    if not meta["skip_g"]:
        g_rep = res.tile([P, D], F32, name="g_rep")
    if not meta["skip_be"]:
        be_rep = res.tile([P, D], F32, name="be_rep")

    nc.sync.dma_start(out=gidx16_sb[:], in_=I["gidx16"][:])
    nc.sync.dma_start(out=dlix16_sb[:], in_=I["dlix16"][:])
    nc.sync.dma_start(out=dlmod_sb[:], in_=I["dlmod"][:])
    for k in range(KD):
        nc.sync.dma_start(out=xT_sb[k][:], in_=I["xT"][k])

    for b in range(NB):
        nc.vector.tensor_tensor(
            out=mk_all[:, b, :, :],
            in0=iota_sb[:].unsqueeze(1).to_broadcast([P, CH, P]),
            in1=dlmod_sb[:, b, :].unsqueeze(2).to_broadcast([P, CH, P]),
            op=ALU.is_equal,
        )
        for ch in range(CH):
            mkT_ps = psum.tile([P, P], BF16, name="mkT_ps", tag="tr", bufs=2)
            nc.tensor.transpose(
                out=mkT_ps[:], in_=mk_all[:, b, ch, :], identity=id128[:]
            )
            nc.vector.tensor_copy(out=mkT_all[:, b, ch, :], in_=mkT_ps[:])

    # ---------------- DRAM comm buffers
    ag_in = dram.tile([SH, RP], BF16, name="ag_in")
    ag_outs = [
        dram.tile([nc_cores * SH, RP], BF16, name=f"ag_out{l}", addr_space=shared)
        for l in range(L)
    ]
    ar_in = dram.tile([G, D + 1], F32, name="ar_in")
    ar_out = dram.tile([G, D + 1], F32, name="ar_out", addr_space=shared)

    for l in range(L):
        ag_out = ag_outs[l]
        # ---------- load layer weights
        for k in range(KD):
            nc.sync.dma_start(out=W_sb[k][:], in_=I["W_all"][l, k])
            nc.sync.dma_start(out=WT_sb[k][:], in_=I["WT_all"][l, k])
            nc.sync.dma_start(out=A_sb[k][:], in_=I["A_all"][l, k])
        if b_rep is not None:
            nc.sync.dma_start(out=b_rep[:], in_=I["b_rep"][l])
        if g_rep is not None:
            nc.sync.dma_start(out=g_rep[:], in_=I["g_rep"][l])
        if be_rep is not None:
            nc.sync.dma_start(out=be_rep[:], in_=I["be_rep"][l])

        # ---------- WA = W @ A   (waT = A^T W^T, then transpose 128-slices)
        waT_ps = psum.tile([H2, D], F32, name="waT_ps", tag="big", bufs=2)
        for k in range(KD):
            nc.tensor.matmul(
                out=waT_ps[:], lhsT=A_sb[k][:], rhs=WT_sb[k][:],
                start=(k == 0), stop=(k == KD - 1),
            )
        nc.scalar.copy(out=waT_sb[:], in_=waT_ps[:])
        for k in range(KD):
            wa_ps = psum.tile([P, H2], BF16, name="wa_ps", tag="tr", bufs=2)
            nc.tensor.transpose(
                out=wa_ps[:], in_=waT_sb[:, k * P : (k + 1) * P], identity=idh2[:]
            )
            nc.scalar.copy(out=wa_sb[k][:], in_=wa_ps[:])

        # ---------- GEMM: h + he per block -> ag_in / edst table
        for b in range(NB):
            h_ps = psum.tile([P, D], F32, name="h_ps", tag="big", bufs=2)
            he_ps = psum.tile([P, H2], F32, name="he_ps", tag="ed", bufs=2)
            for k in range(KD):
                lhsT = xT_sb[k][:, b * P : (b + 1) * P]
                nc.tensor.matmul(
                    out=h_ps[:], lhsT=lhsT, rhs=W_sb[k][:],
                    start=(k == 0), stop=(k == KD - 1),
                )
                nc.tensor.matmul(
                    out=he_ps[:], lhsT=lhsT, rhs=wa_sb[k][:],
                    start=(k == 0), stop=(k == KD - 1),
                )
            nc.scalar.copy(out=hn_sb[b][:], in_=h_ps[:])
            nc.scalar.copy(out=henm_sb[b][:], in_=he_ps[:])
            nc.sync.dma_start(out=ag_in[b * P : (b + 1) * P, 0:D], in_=hn_sb[b][:])
            nc.sync.dma_start(
                out=ag_in[b * P : (b + 1) * P, D : D + H2], in_=henm_sb[b][:]
            )

        # ---------- AllGather [h | e_src]
        if os.environ.get("GAT_NO_AG", "0") == "1":
            # diagnostic ablation: wrong results, same local compute
            nc.sync.dma_start(out=ag_out[0:SH, :], in_=ag_in[:])
        else:
            nc.gpsimd.collective_compute(
                "AllGather",
                ALU.bypass,
                replica_groups=rg,
                ins=[ag_in.opt()],
                outs=[ag_out.opt()],
            )

        # ---------- edge phase
        for b in range(NB):
            # gather [h | e_src] rows by source id (batched SWDGE gather)
            gt = sb.tile([P, CH, RP], BF16, name="gt", tag="gt", bufs=2)
            nc.gpsimd.dma_gather(
                gt[:], ag_out[:], gidx16_sb[:, b, :], CH * P, CH * P, RP,
            )
            # e_dst per edge = mkT.T @ e_dst_block
            ed_ps = psum.tile([P, CH, H], F32, name="ed_ps", tag="ed", bufs=2)
            for ch in range(CH):
                nc.tensor.matmul(
                    out=ed_ps[:, ch, :], lhsT=mkT_all[:, b, ch, :],
                    rhs=henm_sb[b][:, H:H2], start=True, stop=True,
                )
            # e = leaky_relu(e_src + e_dst); ee = exp(e)  (block-batched)
            e_sb = sb.tile([P, CH, H], F32, name="e_sb", tag="e_sb", bufs=2)
            nc.vector.tensor_add(out=e_sb[:], in0=gt[:, :, D:ROW], in1=ed_ps[:])
            nc.vector.scalar_tensor_tensor(
                out=e_sb[:], in0=e_sb[:], scalar=NEG, in1=e_sb[:],
                op0=ALU.mult, op1=ALU.max,
            )
            ee_sb = sb.tile([P, CH, H], BF16, name="ee_sb", tag="ee_sb", bufs=2)
            nc.scalar.activation(ee_sb[:], e_sb[:], ACT.Exp)
            # self-loop terms
            es_sb = sb.tile([P, H], F32, name="es_sb", tag="es_sb", bufs=2)
            nc.vector.tensor_add(
                out=es_sb[:], in0=henm_sb[b][:, 0:H], in1=henm_sb[b][:, H:H2]
            )
            nc.vector.scalar_tensor_tensor(
                out=es_sb[:], in0=es_sb[:], scalar=NEG, in1=es_sb[:],
                op0=ALU.mult, op1=ALU.max,
            )
            eself = sb.tile([P, H], F32, name="eself", tag="eself", bufs=2)
            nc.scalar.activation(eself[:], es_sb[:], ACT.Exp)

            out_ps = psum.tile([P, D], F32, name="out_ps", tag="big", bufs=2)
            den_ps = psum.tile([P, H], F32, name="den_ps", tag="den", bufs=2)
            gs = sb.tile([P, CH, D], BF16, name="gs", tag="gs", bufs=2)
            nc.vector.tensor_tensor(
                out=gs[:].rearrange("p ch (c h) -> p ch c h", h=H),
                in0=gt[:, :, 0:D].rearrange("p ch (c h) -> p ch c h", h=H),
                in1=ee_sb[:].unsqueeze(2).to_broadcast([P, CH, C, H]),
                op=ALU.mult,
            )
            for ch in range(CH):
                nc.tensor.matmul(
                    out=out_ps[:], lhsT=mk_all[:, b, ch, :], rhs=gs[:, ch, :],
                    start=(ch == 0), stop=(ch == CH - 1),
                )
                nc.tensor.matmul(
                    out=den_ps[:], lhsT=mk_all[:, b, ch, :], rhs=ee_sb[:, ch, :],
                    start=(ch == 0), stop=(ch == CH - 1),
                )

            # ----- epilogue: softmax-normalize + self loops, LN stats
            den_sb = sb.tile([P, H], F32, name="den_sb", tag="den_sb", bufs=2)
            nc.vector.tensor_add(out=den_sb[:], in0=den_ps[:], in1=eself[:])
            rec_sb = sb.tile([P, H], F32, name="rec_sb", tag="rec_sb", bufs=2)
            nc.vector.reciprocal(out=rec_sb[:], in_=den_sb[:])
            wself = sb.tile([P, H], BF16, name="wself", tag="wself", bufs=2)
            nc.vector.tensor_mul(out=wself[:], in0=eself[:], in1=rec_sb[:])
            y_sb = sb.tile([P, D], BF16, name="y_sb", tag="y_sb", bufs=2)
            nc.vector.tensor_tensor(
                out=y_sb[:].rearrange("p (c h) -> p c h", h=H),
                in0=out_ps[:].rearrange("p (c h) -> p c h", h=H),
                in1=rec_sb[:].unsqueeze(1).to_broadcast([P, C, H]),
                op=ALU.mult,
            )
            st_sb = sb.tile([P, D], BF16, name="st_sb", tag="st_sb", bufs=2)
            nc.vector.tensor_tensor(
                out=st_sb[:].rearrange("p (c h) -> p c h", h=H),
                in0=hn_sb[b][:].rearrange("p (c h) -> p c h", h=H),
                in1=wself[:].unsqueeze(1).to_broadcast([P, C, H]),
                op=ALU.mult,
            )
            nc.vector.tensor_add(out=ybf_sb[b][:], in0=y_sb[:], in1=st_sb[:])
            if b_rep is not None:
                nc.vector.tensor_add(
                    out=ybf_sb[b][:], in0=ybf_sb[b][:], in1=b_rep[:]
                )
            st6 = sb.tile([P, 6], F32, name="st6", tag="st6", bufs=2)
            nc.vector.bn_stats(out=st6[:], in_=ybf_sb[b][:])
            nc.vector.bn_aggr(out=mv_all[:, b, :], in_=st6[:])

        # ---------- batched rstd = exp(-0.5*ln(var+eps)): 2 ACT calls/layer
        lv_all = sb.tile([P, NB], F32, name="lv_all", tag="lv", bufs=2)
        nc.scalar.activation(
            lv_all[:], mv_all[:, :, 1], ACT.Ln, bias=eps_sb[:, 0:1]
        )
        nc.scalar.activation(rstd_all[:], lv_all[:], ACT.Exp, scale=-0.5)

        # ---------- normalize + relu (+ transpose for next layer)
        for b in range(NB):
            mm = sb.tile([P, 1], F32, name="mm", tag="mm", bufs=2)
            nc.vector.scalar_tensor_tensor(
                out=mm[:], in0=mv_all[:, b, 0:1], scalar=-1.0,
                in1=rstd_all[:, b : b + 1], op0=ALU.mult, op1=ALU.mult,
            )
            if g_rep is None and be_rep is None:
                # in place: xn_sb[b] holds pre-norm y (elementwise stream)
                nc.scalar.activation(
                    xn_sb[b][:], ybf_sb[b][:], ACT.Relu,
                    bias=mm[:, 0:1], scale=rstd_all[:, b : b + 1],
                )
            else:
                ln_sb = sb.tile([P, D], F32, name="ln_sb", tag="ln_sb", bufs=2)
                nc.scalar.activation(
                    ln_sb[:], ybf_sb[b][:], ACT.Identity,
                    bias=mm[:, 0:1], scale=rstd_all[:, b : b + 1],
                )
                if g_rep is not None:
                    nc.vector.tensor_mul(out=ln_sb[:], in0=ln_sb[:], in1=g_rep[:])
                if be_rep is not None:
                    nc.vector.tensor_add(out=ln_sb[:], in0=ln_sb[:], in1=be_rep[:])
                nc.vector.tensor_scalar_max(
                    out=xn_sb[b][:], in0=ln_sb[:], scalar1=0.0
                )
            if l < L - 1:
                for k in range(KD):
                    t_ps = psum.tile([P, P], BF16, name="t_ps", tag="tr", bufs=2)
                    nc.tensor.transpose(
                        out=t_ps[:],
                        in_=xn_sb[b][:, k * P : (k + 1) * P],
                        identity=id128[:],
                    )
                    nc.scalar.copy(
                        out=xT_sb[k][:, b * P : (b + 1) * P], in_=t_ps[:]
                    )

    # ---------------- pooling (mean by graph) + FC
    ones_sb = res.tile([P, 1], BF16, name="ones_sb")
    nc.vector.memset(ones_sb[:], 1.0)
    pm_pool = ctx.enter_context(tc.tile_pool(name="pm", bufs=2))
    pool_ps = psum.tile([G, D], F32, name="pool_ps", tag="big", bufs=2)
    cnt_ps = psum.tile([G, 1], F32, name="cnt_ps", tag="den", bufs=2)
    for b in range(NB):
        pm_sb = pm_pool.tile([P, G], BF16, name="pm_sb", tag="pm_sb", bufs=2)
        nc.sync.dma_start(out=pm_sb[:], in_=I["poolmask"][b])
        nc.tensor.matmul(
            out=pool_ps[:], lhsT=pm_sb[:], rhs=xn_sb[b][:],
            start=(b == 0), stop=(b == NB - 1),
        )
        nc.tensor.matmul(
            out=cnt_ps[:], lhsT=pm_sb[:], rhs=ones_sb[:],
            start=(b == 0), stop=(b == NB - 1),
        )
    pool_sb = res.tile([G, D + 1], F32, name="pool_sb")
    nc.vector.tensor_copy(out=pool_sb[:, 0:D], in_=pool_ps[:])
    nc.vector.tensor_copy(out=pool_sb[:, D : D + 1], in_=cnt_ps[:])
    nc.sync.dma_start(out=ar_in[:], in_=pool_sb[:])
    nc.gpsimd.collective_compute(
        "AllReduce",
        ALU.add,
        replica_groups=rg,
        ins=[ar_in.opt()],
        outs=[ar_out.opt()],
    )
    pf_sb = res.tile([G, D + 1], F32, name="pf_sb")
    nc.sync.dma_start(out=pf_sb[:], in_=ar_out[:])
    cntf = res.tile([G, 1], F32, name="cntf")
    nc.vector.tensor_scalar_max(out=cntf[:], in0=pf_sb[:, D : D + 1], scalar1=1.0)
    crec = res.tile([G, 1], F32, name="crec")
    nc.vector.reciprocal(out=crec[:], in_=cntf[:])
    pn_sb = res.tile([G, D], BF16, name="pn_sb")
    nc.vector.tensor_tensor(
        out=pn_sb[:], in0=pf_sb[:, 0:D],
        in1=crec[:].to_broadcast([G, D]), op=ALU.mult,
    )
    # transpose pooled -> (KD chunks of (128, G))
    pT_sb = res.tile([P, KD, G], BF16, name="pT_sb")
    for k in range(KD):
        t2_ps = psum.tile([P, G], BF16, name="t2_ps", tag="tr", bufs=2)
        nc.tensor.transpose(
            out=t2_ps[:], in_=pn_sb[:, k * P : (k + 1) * P], identity=id128[:]
        )
        nc.scalar.copy(out=pT_sb[:, k, :], in_=t2_ps[:])
    fcw_sb = res.tile([P, KD, OUT], BF16, name="fcw_sb")
    nc.sync.dma_start(
        out=fcw_sb[:], in_=I["fc_W"][:].rearrange("k p o -> p k o")
    )
    fc_ps = psum.tile([G, OUT], F32, name="fc_ps", tag="big", bufs=2)
    for k in range(KD):
        nc.tensor.matmul(
            out=fc_ps[:], lhsT=pT_sb[:, k, :], rhs=fcw_sb[:, k, :],
            start=(k == 0), stop=(k == KD - 1),
        )
    o_sb = res.tile([G, OUT], F32, name="o_sb")
    if not meta["skip_fcb"]:
        fcb_rep = res.tile([P, OUT], F32, name="fcb_rep")
        nc.sync.dma_start(out=fcb_rep[:], in_=I["fcb_rep"][:])
        nc.vector.tensor_add(out=o_sb[:], in0=fc_ps[:], in1=fcb_rep[0:G, :])
    else:
        nc.vector.tensor_copy(out=o_sb[:], in_=fc_ps[:])
    nc.sync.dma_start(out=out_ap[:], in_=o_sb[:])
    ctx.close()


# --------------------------------------------------------------------------
# Entry point
# --------------------------------------------------------------------------


def build_nc(inputs):
    """Compile the kernel; returns (nc, in_maps, cfg)."""
    cfg = _full_cfg()
    in_maps, meta = _prep(inputs, cfg)

    nc = bacc.Bacc(
        "TRN2",
        target_bir_lowering=False,
        debug=False,
        enable_asserts=False,
        num_devices=cfg["n_cores"],
    )
    I = {}
    for name, arr in in_maps[0].items():
        I[name] = nc.dram_tensor(
            name, arr.shape, mybir.dt.from_np(arr.dtype), kind="ExternalInput"
        ).ap()
    out_ap = nc.dram_tensor(
        "out", (cfg["G"], cfg["OUT"]), F32, kind="ExternalOutput"
    ).ap()

    with tile.TileContext(nc) as tc:
        build(tc, cfg, meta, I, out_ap)
    nc.compile()
    return nc, in_maps, cfg


def kernel(**inputs):
    global LAST_RESULTS
    nc, in_maps, cfg = build_nc(inputs)

    trace = bool(int(os.environ.get("GAT_TRACE", "0")))
    res = run_bass_kernel_spmd(
        nc,
        in_maps,
        core_ids=list(range(cfg["n_cores"])),
        trace=trace,
    )
    LAST_RESULTS = res
    return np.asarray(res.results[0]["out"])
